# revision 6
# baseline (speedup 1.0000x reference)
"""Trainium2 Bass kernel for nn_CrossAttention (LoRA cross-attention).

Sharding: data-parallel over batch — 16 batches across 8 cores, 2 per core.

Per-core layout strategy (all feature-major, zero transposes):
  - z arrives [C=64, HW=4096] per batch (native layout) = q-proj rhs.
  - q^T, k^T computed feature-major; v computed key-major [77, 64].
  - Scores computed L-major: all 8 heads' 77-key score blocks are
    bin-packed into 5 groups of <=128 PSUM partitions via a packed
    block-diagonal K stationary, so exp() runs at full 128-lane ACT
    utilization.
  - Softmax sums come free from an augmented block-diagonal V stationary
    (8 indicator columns); normalization is applied to the [64, HW]
    attention output via a PE-expanded reciprocal.
"""

import sys

sys.path.insert(0, "/opt/trn_rl_repo")

import numpy as np

LATENT = 64
COND = 768
HEADS = 8
DH = LATENT // HEADS  # 8
R = 8
SCALING = 1.0
SCALE = (LATENT / HEADS) ** -0.5
B = 16
HW = 4096
L = 77
N_CORES = 8
BPC = B // N_CORES  # 2 batches per core
CHUNK = 512
NCHUNK = HW // CHUNK  # 8
KCH = COND // 128  # 6 contraction chunks for k/v proj


def make_groups():
    """Bin-pack 8 heads x 77 keys into groups of <=128 score rows.

    Returns list of groups; each group is a list of segments
    (h, c0, c1, off): head h's keys [c0, c1) live at group rows
    [off, off + c1 - c0).
    """
    groups = []
    h, c = 0, 0
    while h < HEADS:
        segs = []
        off = 0
        while off < 128 and h < HEADS:
            take = min(L - c, 128 - off)
            segs.append((h, c, c + take, off))
            off += take
            c += take
            if c == L:
                h += 1
                c = 0
        groups.append(segs)
    return groups


GROUPS = make_groups()
NG = len(GROUPS)  # 5
GROUP_ROWS = [sum(c1 - c0 for _, c0, c1, _ in g) for g in GROUPS]  # 128,..,104
DOTS_W = 512  # psum tile width for score chunks
NDOTS = HW // DOTS_W  # 4


def _build_program(reps: int = 1):
    import concourse.mybir as mybir
    import concourse.tile as tile
    from concourse import bacc

    f32 = mybir.dt.float32
    AF = mybir.ActivationFunctionType

    nc = bacc.Bacc("TRN2", target_bir_lowering=False, debug=False,
                   num_devices=N_CORES)

    def din(name, shape):
        return nc.dram_tensor(name, shape, f32, kind="ExternalInput").ap()

    z_in = din("z", [BPC, LATENT, HW])
    cond_in = din("cond", [BPC, L, COND])
    wq2 = din("wq2", [128, 64])       # [Wq_s^T; Wq_s^T]
    bq2 = din("bq2", [128, 1])
    wkt = din("wkt", [128, KCH * 64])  # Wk_eff^T chunked
    bk2 = din("bk2", [128, 1])
    wvt = din("wvt", [128, KCH * 64])  # Wv_eff^T chunked
    bv1 = din("bv1", [1, 64])
    wot = din("wot", [64, 64])         # Wo_eff^T
    bo1 = din("bo1", [64, 1])
    emat = din("emat", [72, 64])       # expansion: emat[64+h, h*8+d] = 1
    vbase = din("vbase", [128, NG * 72])  # indicator cols pattern
    out_d = nc.dram_tensor("out", [BPC, LATENT, HW], f32,
                           kind="ExternalOutput").ap()

    with tile.TileContext(nc) as tc:
        with (
            tc.tile_pool(name="persist", bufs=1) as pp,
            tc.tile_pool(name="work", bufs=2) as wp,
            tc.tile_pool(name="psum", bufs=1, space="PSUM") as psp,
        ):
            # ---- persistent SBUF tensors ----
            x_sb = pp.tile([128, HW], f32)          # z, both batches stacked
            qT_sb = pp.tile([128, HW], f32)
            kT_sb = pp.tile([128, L], f32)
            condT = pp.tile([128, BPC * KCH * L], f32)
            kpack = pp.tile([128, NG * 128], f32)
            wq_sb = pp.tile([128, 64], f32)
            bq_sb = pp.tile([128, 1], f32)
            wkt_sb = pp.tile([128, KCH * 64], f32)
            bk_sb = pp.tile([128, 1], f32)
            wvt_sb = pp.tile([128, KCH * 64], f32)
            bv_sb = pp.tile([1, 64], f32)
            wot_sb = pp.tile([64, 64], f32)
            bo_sb = pp.tile([64, 1], f32)
            emat_sb = pp.tile([72, 64], f32)
            ones_sb = pp.tile([1, L], f32)
            v_sb = [pp.tile([L, 64], f32, name=f"v_sb{b}") for b in range(BPC)]
            vbig = [pp.tile([128, NG * 72], f32, name=f"vbig{b}")
                    for b in range(BPC)]
            exp_sb = [pp.tile([128, HW], f32, name=f"expsb{g}")
                      for g in range(NG)]

            # ---- weight / constant loads ----
            nc.sync.dma_start(out=wq_sb[:], in_=wq2[:])
            nc.sync.dma_start(out=bq_sb[:], in_=bq2[:])
            nc.sync.dma_start(out=wkt_sb[:], in_=wkt[:])
            nc.sync.dma_start(out=bk_sb[:], in_=bk2[:])
            nc.sync.dma_start(out=wvt_sb[:], in_=wvt[:])
            nc.sync.dma_start(out=bv_sb[:], in_=bv1[:])
            nc.sync.dma_start(out=wot_sb[:], in_=wot[:])
            nc.sync.dma_start(out=bo_sb[:], in_=bo1[:])
            nc.sync.dma_start(out=emat_sb[:], in_=emat[:])
            nc.vector.memset(ones_sb[:], 1.0)

            for _rep in range(reps):
                # ---- input loads ----
                for b in range(BPC):
                    nc.sync.dma_start(out=x_sb[b * 64:(b + 1) * 64, :],
                                      in_=z_in[b])
                    condr = cond_in[b].rearrange("l (j f) -> j f l", f=128)
                    for j in range(KCH):
                        s = (b * KCH + j) * L
                        nc.sync.dma_start(out=condT[:, s:s + L], in_=condr[j])
                    nc.sync.dma_start(out=vbig[b][:], in_=vbase[:])

                # ---- q projection (both batches, quadrant-parallel) ----
                for c in range(NCHUNK):
                    qps = psp.tile([128, CHUNK], f32, tag="qps", bufs=2)
                    cs = slice(c * CHUNK, (c + 1) * CHUNK)
                    for b in range(BPC):
                        p = slice(b * 64, (b + 1) * 64)
                        nc.tensor.matmul(out=qps[p, :], lhsT=wq_sb[p, :],
                                         rhs=x_sb[p, cs])
                    nc.scalar.activation(out=qT_sb[:, cs], in_=qps[:],
                                         func=AF.Identity, bias=bq_sb[:])

                # ---- k projection ----
                kps = psp.tile([128, L], f32, tag="qps", bufs=2)
                for b in range(BPC):
                    p = slice(b * 64, (b + 1) * 64)
                    for j in range(KCH):
                        s = (b * KCH + j) * L
                        nc.tensor.matmul(out=kps[p, :],
                                         lhsT=wkt_sb[:, j * 64:(j + 1) * 64],
                                         rhs=condT[:, s:s + L],
                                         start=(j == 0), stop=(j == KCH - 1))
                nc.scalar.activation(out=kT_sb[:], in_=kps[:],
                                     func=AF.Identity, bias=bk_sb[:])

                # ---- v projection (key-major) + bias via ones-row ----
                for b in range(BPC):
                    vps = psp.tile([L, 64], f32, tag="qps", bufs=2)
                    for j in range(KCH):
                        s = (b * KCH + j) * L
                        nc.tensor.matmul(out=vps[:],
                                         lhsT=condT[:, s:s + L],
                                         rhs=wvt_sb[:, j * 64:(j + 1) * 64],
                                         start=(j == 0), stop=False)
                    nc.tensor.matmul(out=vps[:], lhsT=ones_sb[:],
                                     rhs=bv_sb[:], start=False, stop=True)
                    nc.vector.tensor_copy(out=v_sb[b][:], in_=vps[:])

                # ---- pack K into block-diagonal group stationaries ----
                nc.vector.memset(kpack[:], 0.0)
                for b in range(BPC):
                    for g, segs in enumerate(GROUPS):
                        for (h, c0, c1, off) in segs:
                            p = slice(b * 64 + h * 8, b * 64 + h * 8 + 8)
                            nc.sync.dma_start(
                                out=kpack[p, g * 128 + off:g * 128 + off + (c1 - c0)],
                                in_=kT_sb[p, c0:c1])
                # ---- scatter v into block-diagonal aug-V stationaries ----
                for b in range(BPC):
                    for g, segs in enumerate(GROUPS):
                        for (h, c0, c1, off) in segs:
                            nc.sync.dma_start(
                                out=vbig[b][off:off + (c1 - c0),
                                            g * 72 + h * 8:g * 72 + h * 8 + 8],
                                in_=v_sb[b][c0:c1, h * 8:h * 8 + 8])

                # ---- per batch: scores+exp, then AV+normalize+o-proj ----
                for b in range(BPC):
                    p = slice(b * 64, (b + 1) * 64)
                    # phase A: dots + exp
                    for g in range(NG):
                        rg = GROUP_ROWS[g]
                        for dc in range(NDOTS):
                            dps = psp.tile([128, DOTS_W], f32, tag="dots", bufs=2)
                            for i in range(DOTS_W // CHUNK):
                                pos = dc * DOTS_W + i * CHUNK
                                nc.tensor.matmul(
                                    out=dps[0:rg, i * CHUNK:(i + 1) * CHUNK],
                                    lhsT=kpack[p, g * 128:g * 128 + rg],
                                    rhs=qT_sb[p, pos:pos + CHUNK])
                            nc.scalar.activation(
                                out=exp_sb[g][0:rg, dc * DOTS_W:(dc + 1) * DOTS_W],
                                in_=dps[0:rg, :], func=AF.Exp)
                    # phase B: AV, normalize, o-proj, store
                    for c in range(NCHUNK):
                        cs = slice(c * CHUNK, (c + 1) * CHUNK)
                        avps = psp.tile([72, CHUNK], f32, tag="avps", bufs=2)
                        for g in range(NG):
                            rg = GROUP_ROWS[g]
                            nc.tensor.matmul(
                                out=avps[:],
                                lhsT=vbig[b][0:rg, g * 72:(g + 1) * 72],
                                rhs=exp_sb[g][0:rg, cs],
                                start=(g == 0), stop=(g == NG - 1))
                        nrm = wp.tile([72, CHUNK], f32, tag="nrm")
                        nc.vector.reciprocal(out=nrm[64:72, :],
                                             in_=avps[64:72, :])
                        eps = psp.tile([64, CHUNK], f32, tag="eps", bufs=2)
                        nc.tensor.matmul(out=eps[:], lhsT=emat_sb[64:72, :],
                                         rhs=nrm[64:72, :])
                        esb = wp.tile([64, CHUNK], f32, tag="esb")
                        nc.vector.tensor_copy(out=esb[:], in_=eps[:])
                        nrmd = wp.tile([64, CHUNK], f32, tag="nrmd")
                        nc.vector.tensor_mul(out=nrmd[:], in0=avps[0:64, :],
                                             in1=esb[:])
                        ops = psp.tile([64, CHUNK], f32, tag="eps", bufs=2)
                        nc.tensor.matmul(out=ops[:], lhsT=wot_sb[:],
                                         rhs=nrmd[:])
                        osb = wp.tile([64, CHUNK], f32, tag="osb", bufs=3)
                        nc.vector.tensor_scalar_add(out=osb[:], in0=ops[:],
                                                    scalar1=bo_sb[:])
                        nc.sync.dma_start(out=out_d[b, :, cs], in_=osb[:])
    nc.compile()
    return nc


def _prep_weights(Wq, bq, Aq, Bq, Wk, bk, Ak, Bk, Wv, bv, Av, Bv,
                  Wo, bo, Ao, Bo):
    def eff(W, A, Bm):
        return (W + SCALING * (Bm @ A)).astype(np.float32)

    Wq_s = eff(Wq, Aq, Bq) * SCALE
    bq_s = (bq * SCALE).astype(np.float32)
    Wk_e, Wv_e, Wo_e = eff(Wk, Ak, Bk), eff(Wv, Av, Bv), eff(Wo, Ao, Bo)

    def chunked_T(We):  # [64, 768] -> [128, 6*64]
        WT = We.T.reshape(KCH, 128, 64)
        return np.ascontiguousarray(
            WT.transpose(1, 0, 2).reshape(128, KCH * 64)).astype(np.float32)

    emat = np.zeros((72, 64), np.float32)
    for h in range(HEADS):
        for d in range(DH):
            emat[64 + h, h * 8 + d] = 1.0
    vbase = np.zeros((128, NG * 72), np.float32)
    for g, segs in enumerate(GROUPS):
        for (h, c0, c1, off) in segs:
            vbase[off:off + (c1 - c0), g * 72 + 64 + h] = 1.0
    return {
        "wq2": np.concatenate([Wq_s.T, Wq_s.T], 0).astype(np.float32),
        "bq2": np.concatenate([bq_s, bq_s])[:, None].astype(np.float32),
        "wkt": chunked_T(Wk_e),
        "bk2": np.concatenate([bk, bk])[:, None].astype(np.float32),
        "wvt": chunked_T(Wv_e),
        "bv1": bv[None, :].astype(np.float32),
        "wot": np.ascontiguousarray(Wo_e.T).astype(np.float32),
        "bo1": bo[:, None].astype(np.float32),
        "emat": emat,
        "vbase": vbase,
    }


class _Runner:
    """Builds the sharded jit once; supports repeated timed executions."""

    def __init__(self, nc, n_cores):
        import jax
        import concourse.mybir as mybir
        from jax.sharding import Mesh, PartitionSpec
        from jax.experimental.shard_map import shard_map
        from concourse import bass2jax
        from concourse.bass2jax import _bass_exec_p, install_neuronx_cc_hook

        install_neuronx_cc_hook()
        self.jax = jax
        self.nc = nc
        self.n = n_cores
        pname = nc.partition_id_tensor.name if nc.partition_id_tensor else None
        in_names, out_names, out_avals, zeros = [], [], [], []
        for alloc in nc.m.functions[0].allocations:
            if not isinstance(alloc, mybir.MemoryLocationSet):
                continue
            name = alloc.memorylocations[0].name
            if alloc.kind == "ExternalInput":
                if name != pname:
                    in_names.append(name)
            elif alloc.kind == "ExternalOutput":
                out_names.append(name)
                shape = tuple(alloc.tensor_shape)
                dt = mybir.dt.np(alloc.dtype)
                out_avals.append(jax.core.ShapedArray(shape, dt))
                zeros.append(np.zeros(shape, dt))
        self.in_names, self.out_names = in_names, out_names
        self.out_avals, self.zeros = out_avals, zeros
        all_in = in_names + out_names + ([pname] if pname else [])

        def _body(*args):
            ops = list(args)
            if pname:
                ops.append(bass2jax.partition_id_tensor())
            return tuple(_bass_exec_p.bind(
                *ops, out_avals=tuple(out_avals), in_names=tuple(all_in),
                out_names=tuple(out_names), lowering_input_output_aliases=(),
                sim_require_finite=True, sim_require_nnan=True, nc=nc))

        devices = jax.devices()[:n_cores]
        mesh = Mesh(np.asarray(devices), ("core",))
        nin = len(in_names) + len(zeros)
        self.fn = jax.jit(
            shard_map(_body, mesh=mesh, in_specs=(PartitionSpec("core"),) * nin,
                      out_specs=(PartitionSpec("core"),) * len(out_names),
                      check_rep=False),
            keep_unused=True)
        self._dev = None

    def set_inputs(self, in_maps):
        jax, n = self.jax, self.n
        cat = [np.concatenate([np.asarray(in_maps[c][nm]) for c in range(n)], 0)
               for nm in self.in_names]
        catz = [np.zeros((n * z.shape[0], *z.shape[1:]), z.dtype)
                for z in self.zeros]
        self._dev = [jax.device_put(a) for a in cat + catz]

    def run(self):
        out = self.fn(*self._dev)
        self.jax.block_until_ready(out)
        return out

    def results(self, out):
        n = self.n
        return [{nm: np.asarray(out[i]).reshape(n, *self.out_avals[i].shape)[c]
                 for i, nm in enumerate(self.out_names)}
                for c in range(n)]


_STATE = {}


def _get_runner(reps: int = 1):
    key = ("runner", reps)
    if key not in _STATE:
        nc = _build_program(reps)
        _STATE[key] = _Runner(nc, N_CORES)
    return _STATE[key]


def kernel(z, cond, Wq, bq, Aq, Bq, Wk, bk, Ak, Bk, Wv, bv, Av, Bv,
           Wo, bo, Ao, Bo):
    z = np.asarray(z, np.float32)
    cond = np.asarray(cond, np.float32)
    w = _prep_weights(np.asarray(Wq), np.asarray(bq), np.asarray(Aq),
                      np.asarray(Bq), np.asarray(Wk), np.asarray(bk),
                      np.asarray(Ak), np.asarray(Bk), np.asarray(Wv),
                      np.asarray(bv), np.asarray(Av), np.asarray(Bv),
                      np.asarray(Wo), np.asarray(bo), np.asarray(Ao),
                      np.asarray(Bo))
    r = _get_runner()
    in_maps = []
    for c in range(N_CORES):
        m = dict(w)
        m["z"] = np.ascontiguousarray(
            z[c * BPC:(c + 1) * BPC].reshape(BPC, LATENT, HW))
        m["cond"] = np.ascontiguousarray(cond[c * BPC:(c + 1) * BPC])
        in_maps.append(m)
    r.set_inputs(in_maps)
    res = r.results(r.run())
    out = np.empty((B, LATENT, 64, 64), np.float32)
    for c in range(N_CORES):
        out[c * BPC:(c + 1) * BPC] = res[c]["out"].reshape(BPC, LATENT, 64, 64)
    return out
